# revision 25
# baseline (speedup 1.0000x reference)
"""Conv2d(256->256, 3x3, pad=1) on 8 TRN2 NeuronCores -- F(4,3) variant.

Same structure as the F(2,3) kernel but 1D Winograd F(4,3) along W:
6 points per 4 outputs -> 4.5 MACs/output instead of 6 -> 288 matmuls of
N=392 (47 us stream) instead of 336 of N=448 (62.7 us). bf16 rel err
~9.4e-3 (validated numerically; gate is 2e-2).

Points {0,+-1,+-2,inf}; host applies B^T/G transforms, device accumulates
M[p] = sum_(cb,kh) W[p,kh]^T X[p] into 6 PSUM banks per (h-group,
o-block) tile (HR=7 rows, N=7*56=392), casts to bf16 (Vector/Scalar
alternating), host applies A^T.
"""

import sys

sys.path.insert(0, "/opt/trn_rl_repo")

import numpy as np
import ml_dtypes

import concourse.mybir as mybir
from concourse import bacc
from concourse.tile import TileContext
from concourse.bass_utils import run_bass_kernel_spmd

N_CORES = 8
C, H, W = 256, 224, 224
O = 256
KH = KW = 3
HS = H // N_CORES          # 28 output rows per core
HR = 7                     # output rows per PSUM tile (N = 7*56 = 392)
J = W // 4                 # 56 Winograd windows per row
NP = 6                     # Winograd points per window
CB = C // 128              # c blocks
OB = O // 128              # o blocks

_CACHE = {}
LAST_RESULTS = None
TRACE = False

BT = np.array([
    [4,  0, -5,  0, 1, 0],
    [0, -4, -4,  1, 1, 0],
    [0,  4, -4, -1, 1, 0],
    [0, -2, -1,  2, 1, 0],
    [0,  2, -1, -2, 1, 0],
    [0,  4,  0, -5, 0, 1],
], dtype=np.float64)
G = np.array([
    [1 / 4,      0,     0],
    [-1 / 6, -1 / 6, -1 / 6],
    [-1 / 6,  1 / 6, -1 / 6],
    [1 / 24, 1 / 12,  1 / 6],
    [1 / 24, -1 / 12, 1 / 6],
    [0,          0,     1],
], dtype=np.float64)
AT = np.array([
    [1, 1,  1, 1,  1, 0],
    [0, 1, -1, 2, -2, 0],
    [0, 1,  1, 4,  4, 0],
    [0, 1, -1, 8, -8, 1],
], dtype=np.float64)


def _build():
    nc = bacc.Bacc(None, target_bir_lowering=False)

    xs = nc.dram_tensor(
        "xs", [CB, 128, HS + 2, NP * J], mybir.dt.bfloat16, kind="ExternalInput"
    )
    w = nc.dram_tensor(
        "w", [CB, OB, 128, NP * KH, 128], mybir.dt.bfloat16, kind="ExternalInput"
    )
    mout = nc.dram_tensor(
        "mout", [OB, 128, HS, NP * J], mybir.dt.bfloat16, kind="ExternalOutput"
    )
    # Last tile lands p-major so each per-p piece is contiguous per
    # partition (784B lines); the row-major mout slice would be 112B lines
    # and its ~0.6 MB would take ~6 us after the last matmul.
    mlast = nc.dram_tensor(
        "mlast", [128, NP, HR, J], mybir.dt.bfloat16, kind="ExternalOutput"
    )

    n_warm = 46
    with TileContext(nc) as tc:
        with (
            tc.tile_pool(name="warm", bufs=1) as pwarm,
            tc.tile_pool(name="win", bufs=1) as pw,
            tc.tile_pool(name="xin", bufs=1) as px,
            tc.tile_pool(name="psumw", bufs=1, space="PSUM") as ppw,
            tc.tile_pool(name="psum", bufs=7, space="PSUM") as pp,
            tc.tile_pool(name="outp", bufs=4) as po,
        ):
            # PE warmup: short N=128 matmuls on a memset tile, sized to
            # keep the PE busy until real operands land.
            wt0 = pwarm.tile([128, 128], mybir.dt.bfloat16, tag="warm")
            ps0 = ppw.tile([128, 128], mybir.dt.float32, tag="warmps")
            nc.vector.memset(wt0[:], 0.0)
            for _ in range(n_warm):
                nc.tensor.matmul(ps0[:], wt0[:], wt0[:], start=True, stop=True)

            x_sb = [
                px.tile(
                    [128, HS + 2, NP * J], mybir.dt.bfloat16,
                    tag=f"x{b}", name=f"x{b}"
                )
                for b in range(CB)
            ]
            w_sb = [
                [
                    pw.tile(
                        [128, NP * KH, 128], mybir.dt.bfloat16,
                        tag=f"w{b}{ob}", name=f"w{b}{ob}"
                    )
                    for ob in range(OB)
                ]
                for b in range(CB)
            ]

            def dma_w(b, ob, k0=0, k1=NP * KH):
                nc.sync.dma_start(
                    out=w_sb[b][ob][:, k0:k1, :], in_=w[b, ob, :, k0:k1, :]
                )

            def dma_x(b, r0, r1):
                nc.sync.dma_start(
                    out=x_sb[b][:, r0:r1, :], in_=xs[b, :, r0:r1, :]
                )

            # Issue order == consumption order of the cb0-half-first head;
            # weights split (p0 / p1-2 / p3-5) so each cb half-pass starts
            # as soon as its first point's taps and x rows are in.
            dma_w(0, 0, 0, 3)
            dma_x(0, 0, 9)
            dma_w(0, 0, 3, 9)
            dma_w(0, 0, 9, NP * KH)
            dma_w(1, 0, 0, 3)
            dma_x(1, 0, 9)
            dma_w(1, 0, 3, 9)
            dma_w(1, 0, 9, NP * KH)
            dma_x(0, 9, 16)
            dma_x(1, 9, 16)
            dma_x(0, 16, 23)
            dma_x(1, 16, 23)
            dma_x(0, 23, 30)
            dma_x(1, 23, 30)
            dma_w(0, 1)
            dma_w(1, 1)

            def mm_half(ps, h0, ob, p, b, first, last):
                for kh in range(KH):
                    nc.tensor.matmul(
                        ps[:],
                        w_sb[b][ob][:, p * KH + kh, :],
                        x_sb[b][:, h0 + kh : h0 + kh + HR, p * J : (p + 1) * J],
                        start=(first and kh == 0),
                        stop=(last and kh == KH - 1),
                    )

            def evac(mo, ps, p, fin=False, eng=None):
                if fin:
                    # p-major staging tile: piece is contiguous per partition.
                    dst = mo[:, p]
                else:
                    dst = mo[:, :, p * J : (p + 1) * J]
                if eng is None:
                    eng = "v" if p % 2 == 0 else "s"
                if eng == "v":
                    nc.vector.tensor_copy(out=dst, in_=ps[:])
                else:
                    nc.scalar.copy(out=dst, in_=ps[:])
                if fin:
                    nc.sync.dma_start(out=mlast[:, p], in_=mo[:, p])

            # First tile: all six cb0 half-groups first (needs only x rows
            # 0..8 of cb0 + w(0,0)), then the cb1 halves.
            mo0 = po.tile([128, HR, NP * J], mybir.dt.bfloat16, tag="mo", name="mo")
            ps0l = []
            for p in range(NP):
                ps = pp.tile([128, HR, J], mybir.dt.float32, tag="ps", name="ps")
                ps0l.append(ps)
                mm_half(ps, 0, 0, p, 0, first=True, last=False)
            for p in range(NP):
                ps = ps0l[p]
                mm_half(ps, 0, 0, p, 1, first=False, last=True)
                evac(mo0, ps, p)
            nc.sync.dma_start(out=mout[0, :, 0:HR, :], in_=mo0[:])

            last = (OB - 1, HS - HR)
            for ob in range(OB):
                for h0 in range(0, HS, HR):
                    if ob == 0 and h0 < HR:
                        continue
                    is_last = (ob, h0) == last
                    if is_last:
                        mo = po.tile(
                            [128, NP, HR, J], mybir.dt.bfloat16,
                            tag="mo", name="mo"
                        )
                    else:
                        mo = po.tile(
                            [128, HR, NP * J], mybir.dt.bfloat16,
                            tag="mo", name="mo"
                        )
                    for p in range(NP):
                        if is_last and p == NP - 1:
                            continue
                        ps = pp.tile(
                            [128, HR, J], mybir.dt.float32, tag="ps", name="ps"
                        )
                        mm_half(ps, h0, ob, p, 0, first=True, last=False)
                        mm_half(ps, h0, ob, p, 1, first=False, last=True)
                        # In the last tile keep VectorE free for the final
                        # point: its p4 cast goes to ScalarE instead.
                        evac(mo, ps, p, fin=is_last,
                             eng="s" if (is_last and p == 4) else None)
                    if not is_last:
                        nc.sync.dma_start(
                            out=mout[ob, :, h0 : h0 + HR, :], in_=mo[:]
                        )
                    else:
                        # Final point p5 split by rows so only a 1-row group's
                        # cast + DMA sits after the very last matmul. The
                        # rows 0..5 piece goes cast(Scalar) -> DMA(Scalar)
                        # while the last 6 matmuls run; the 1-row piece goes
                        # cast(Vector) -> DMA(Scalar) -- ScalarE is an HWDGE
                        # engine, so nothing queues behind SyncE's backlog.
                        p5 = NP - 1
                        for r0, r1 in ((0, HR - 1), (HR - 1, HR)):
                            ps = pp.tile(
                                [128, HR, J], mybir.dt.float32,
                                tag="ps", name="ps"
                            )
                            sub = ps[:, r0:r1, :]
                            for b in range(CB):
                                for kh in range(KH):
                                    nc.tensor.matmul(
                                        sub,
                                        w_sb[b][ob][:, p5 * KH + kh, :],
                                        x_sb[b][
                                            :,
                                            h0 + kh + r0 : h0 + kh + r1,
                                            p5 * J : (p5 + 1) * J,
                                        ],
                                        start=(b == 0 and kh == 0),
                                        stop=(b == CB - 1 and kh == KH - 1),
                                    )
                            dst = mo[:, p5, r0:r1, :]
                            nc.vector.tensor_copy(out=dst, in_=sub)
                            if r1 == HR:
                                # Only this 1-row piece sits after the last
                                # matmul; ScalarE's HWDGE queue is free.
                                nc.scalar.dma_start(
                                    out=mlast[:, p5, r0:r1], in_=dst
                                )
                            else:
                                nc.sync.dma_start(
                                    out=mlast[:, p5, r0:r1], in_=dst
                                )

    nc.compile()
    return nc


def _to_bf16(a):
    return np.ascontiguousarray(a.astype(ml_dtypes.bfloat16))


def kernel(x: np.ndarray, kernel: np.ndarray) -> np.ndarray:
    global LAST_RESULTS
    if "nc" not in _CACHE:
        _CACHE["nc"] = _build()
    nc = _CACHE["nc"]

    x = np.ascontiguousarray(x, dtype=np.float32)
    g = np.ascontiguousarray(kernel, dtype=np.float32)

    xp = np.pad(x, ((0, 0), (1, 1), (1, 1)))          # [C, H+2, 226]
    # Winograd F(4,3) input transform: 6 point-planes x 56 windows.
    Xt = np.zeros((C, H + 2, NP, J), dtype=np.float32)
    for p in range(NP):
        for i in range(6):
            c = BT[p, i]
            if c:
                Xt[:, :, p, :] += np.float32(c) * xp[:, :, i : 4 * (J - 1) + i + 1 : 4]
    Xt = _to_bf16(Xt.reshape(CB, 128, H + 2, NP * J))

    # Weight transform: Wt[p][o, c, kh] = sum_k G[p,k] g[o,c,kh,k].
    gt = g.transpose(1, 2, 3, 0).astype(np.float64)   # [c, kh, kw, o]
    Wt = np.einsum('pk,chko->cpho', G, gt).astype(np.float32)  # [c, p, kh, o]
    # -> [cb, ob, 128 c, p*3+kh, 128 o]
    w_t = _to_bf16(
        Wt.reshape(CB, 128, NP * KH, OB, 128).transpose(0, 3, 1, 2, 4)
    )

    in_maps = []
    for i in range(N_CORES):
        xs_i = np.ascontiguousarray(Xt[:, :, i * HS : i * HS + HS + 2, :])
        in_maps.append({"xs": xs_i, "w": w_t})

    last_err = None
    for _ in range(3):
        try:
            results = run_bass_kernel_spmd(
                nc, in_maps, core_ids=list(range(N_CORES)), trace=TRACE
            )
            break
        except Exception as e:  # noqa: BLE001
            last_err = e
    else:
        raise last_err
    LAST_RESULTS = results

    # Host output transform: y[4j+m] = sum_p AT[m,p] M[p][j].
    out = np.empty((O, H, W), dtype=np.float32)
    for i, r in enumerate(results.results):
        M = r["mout"].reshape(O, HS, NP, J).astype(np.float32)
        # Fold the p-major last tile back in: mlast [128, NP, HR, J] holds
        # (ob=1, rows HS-HR..HS) for this core.
        M[O - 128 :, HS - HR :, :, :] = (
            r["mlast"].transpose(0, 2, 1, 3).astype(np.float32)
        )
        sl = out[:, i * HS : (i + 1) * HS, :]
        for m in range(4):
            acc = np.zeros((O, HS, J), dtype=np.float32)
            for p in range(NP):
                c = AT[m, p]
                if c:
                    acc += np.float32(c) * M[:, :, p, :]
            sl[:, :, m::4] = acc
    return out


# revision 28
# speedup vs baseline: 1.0388x; 1.0388x over previous
"""Conv2d(256->256, 3x3, pad=1) on 8 TRN2 NeuronCores.

Sharding: data-parallel over output rows (H). Each core computes all 256
output channels for a 28-row slice; weights are replicated (the PE
stationary dim stays a full 128 o-channels either way, but H-sharding
needs no output collectives).

Algorithm: 1D Winograd F(4,3) along W (points {0,+-1,+-2,inf}), direct
3-tap accumulation along H, bf16 matmuls: 6 Winograd points per 4
outputs -> 4.5 MACs/output vs 9 direct. The host applies the B^T input
transform (6 point-planes x 56 stride-4 windows per padded row) and the
G weight transform; the device accumulates, per point p, M[p] =
sum_(c-block, kh) W[p,kh]^T X[p] -- 6 accumulating bf16 matmuls into a
PSUM bank [128 o, 7 h-rows x 56 windows = 392] -- then casts PSUM to
bf16 M planes (Vector/Scalar engines); the host applies A^T. 288
matmuls of N=392 per core = 47 us of PE streaming at 2.4 GHz (vs 504
f32r matmuls with ~190 ns exposed self-weight-loads = ~105 us in the
direct-conv baseline). bf16 rel err ~9.6e-3 on HW (gate 2e-2); fp32
PSUM accumulation, fp64 host transforms.

Schedule (from perfetto/NTFF analysis): engine preamble barrier ends
~7 us and DMA data cannot start flowing before ~8 us (fixed runtime
latencies), streaming at ~0.3 GB/us. The head is therefore
arrival-ordered: w(cb,ob=0) in p-split pieces, x rows in one 9-row piece
per c-block, and tile0 runs its six cb0 half-groups first (PSUM groups
stay open) so the PE can start on ~1.4 MB instead of ~2.7 MB.
Dependency-light warmup matmuls (memset tile) cover 7..12.5 us so the
HAM clock-gate is at 8/8 when real work starts, with filler warmups
between tile0's halves so a slow-DMA run cannot idle the PE >2 us and
re-throttle it. PSUM is evacuated per-p right after each group closes,
alternating Vector/Scalar; the final tile lands p-major in a separate
DRAM tensor (contiguous 784B lines) and its last point is row-split so
only a 1-row cast + ScalarE-issued DMA sits after the last matmul.
"""

import sys

sys.path.insert(0, "/opt/trn_rl_repo")

import numpy as np
import ml_dtypes

import concourse.mybir as mybir
from concourse import bacc
from concourse.tile import TileContext
from concourse.bass_utils import run_bass_kernel_spmd

N_CORES = 8
C, H, W = 256, 224, 224
O = 256
KH = KW = 3
HS = H // N_CORES          # 28 output rows per core
HR = 7                     # output rows per PSUM tile (N = 7*56 = 392)
J = W // 4                 # 56 Winograd windows per row
NP = 6                     # Winograd points per window
CB = C // 128              # c blocks
OB = O // 128              # o blocks

_CACHE = {}
LAST_RESULTS = None
TRACE = False

BT = np.array([
    [4,  0, -5,  0, 1, 0],
    [0, -4, -4,  1, 1, 0],
    [0,  4, -4, -1, 1, 0],
    [0, -2, -1,  2, 1, 0],
    [0,  2, -1, -2, 1, 0],
    [0,  4,  0, -5, 0, 1],
], dtype=np.float64)
G = np.array([
    [1 / 4,      0,     0],
    [-1 / 6, -1 / 6, -1 / 6],
    [-1 / 6,  1 / 6, -1 / 6],
    [1 / 24, 1 / 12,  1 / 6],
    [1 / 24, -1 / 12, 1 / 6],
    [0,          0,     1],
], dtype=np.float64)
AT = np.array([
    [1, 1,  1, 1,  1, 0],
    [0, 1, -1, 2, -2, 0],
    [0, 1,  1, 4,  4, 0],
    [0, 1, -1, 8, -8, 1],
], dtype=np.float64)


def _build():
    nc = bacc.Bacc(None, target_bir_lowering=False)

    xs = nc.dram_tensor(
        "xs", [CB, 128, HS + 2, NP * J], mybir.dt.bfloat16, kind="ExternalInput"
    )
    w = nc.dram_tensor(
        "w", [CB, OB, 128, NP * KH, 128], mybir.dt.bfloat16, kind="ExternalInput"
    )
    mout = nc.dram_tensor(
        "mout", [OB, 128, HS, NP * J], mybir.dt.bfloat16, kind="ExternalOutput"
    )
    # Last tile lands p-major so each per-p piece is contiguous per
    # partition (784B lines); the row-major mout slice would be 112B lines
    # and its ~0.6 MB would take ~6 us after the last matmul.
    mlast = nc.dram_tensor(
        "mlast", [128, NP, HR, J], mybir.dt.bfloat16, kind="ExternalOutput"
    )

    n_warm = 50
    with TileContext(nc) as tc:
        with (
            tc.tile_pool(name="warm", bufs=1) as pwarm,
            tc.tile_pool(name="win", bufs=1) as pw,
            tc.tile_pool(name="xin", bufs=1) as px,
            tc.tile_pool(name="psumw", bufs=1, space="PSUM") as ppw,
            tc.tile_pool(name="psum", bufs=7, space="PSUM") as pp,
            tc.tile_pool(name="outp", bufs=4) as po,
        ):
            # PE warmup: short N=128 matmuls on a memset tile, sized to
            # keep the PE busy until real operands land.
            wt0 = pwarm.tile([128, 128], mybir.dt.bfloat16, tag="warm")
            ps0 = ppw.tile([128, 128], mybir.dt.float32, tag="warmps")
            nc.vector.memset(wt0[:], 0.0)
            for _ in range(n_warm):
                nc.tensor.matmul(ps0[:], wt0[:], wt0[:], start=True, stop=True)

            x_sb = [
                px.tile(
                    [128, HS + 2, NP * J], mybir.dt.bfloat16,
                    tag=f"x{b}", name=f"x{b}"
                )
                for b in range(CB)
            ]
            w_sb = [
                [
                    pw.tile(
                        [128, NP * KH, 128], mybir.dt.bfloat16,
                        tag=f"w{b}{ob}", name=f"w{b}{ob}"
                    )
                    for ob in range(OB)
                ]
                for b in range(CB)
            ]

            def dma_w(b, ob, k0=0, k1=NP * KH):
                nc.sync.dma_start(
                    out=w_sb[b][ob][:, k0:k1, :], in_=w[b, ob, :, k0:k1, :]
                )

            def dma_x(b, r0, r1):
                nc.sync.dma_start(
                    out=x_sb[b][:, r0:r1, :], in_=xs[b, :, r0:r1, :]
                )

            # Issue order == consumption order of the cb0-half-first head;
            # weights split (p0 / p1-2 / p3-5) so each cb half-pass starts
            # as soon as its first point's taps and x rows are in.
            dma_w(0, 0, 0, 3)
            dma_x(0, 0, 9)
            dma_w(0, 0, 3, 9)
            dma_w(0, 0, 9, NP * KH)
            dma_w(1, 0, 0, 3)
            dma_x(1, 0, 9)
            dma_w(1, 0, 3, 9)
            dma_w(1, 0, 9, NP * KH)
            dma_x(0, 9, 16)
            dma_x(1, 9, 16)
            dma_x(0, 16, 23)
            dma_x(1, 16, 23)
            dma_x(0, 23, 30)
            dma_x(1, 23, 30)
            dma_w(0, 1)
            dma_w(1, 1)

            def mm_half(ps, h0, ob, p, b, first, last):
                for kh in range(KH):
                    nc.tensor.matmul(
                        ps[:],
                        w_sb[b][ob][:, p * KH + kh, :],
                        x_sb[b][:, h0 + kh : h0 + kh + HR, p * J : (p + 1) * J],
                        start=(first and kh == 0),
                        stop=(last and kh == KH - 1),
                    )

            def evac(mo, ps, p, fin=False, eng=None):
                if fin:
                    # p-major staging tile: piece is contiguous per partition.
                    dst = mo[:, p]
                else:
                    dst = mo[:, :, p * J : (p + 1) * J]
                if eng is None:
                    eng = "v" if p % 2 == 0 else "s"
                if eng == "v":
                    nc.vector.tensor_copy(out=dst, in_=ps[:])
                else:
                    nc.scalar.copy(out=dst, in_=ps[:])
                if fin:
                    nc.sync.dma_start(out=mlast[:, p], in_=mo[:, p])

            # First tile: all six cb0 half-groups first (needs only x rows
            # 0..8 of cb0 + w(0,0)), then the cb1 halves.
            mo0 = po.tile([128, HR, NP * J], mybir.dt.bfloat16, tag="mo", name="mo")
            ps0l = []
            for p in range(NP):
                ps = pp.tile([128, HR, J], mybir.dt.float32, tag="ps", name="ps")
                ps0l.append(ps)
                mm_half(ps, 0, 0, p, 0, first=True, last=False)
            # Filler warmups: if cb1's operands are still in flight, these
            # keep the PE busy through the gap so the HAM clock-gate cannot
            # re-throttle (observed: a ~2 us idle here dropped it to 4/8 and
            # the next ~15 matmuls ran at 1.2 GHz).
            for _ in range(12):
                nc.tensor.matmul(ps0[:], wt0[:], wt0[:], start=True, stop=True)
            for p in range(NP):
                ps = ps0l[p]
                mm_half(ps, 0, 0, p, 1, first=False, last=True)
                evac(mo0, ps, p)
            nc.sync.dma_start(out=mout[0, :, 0:HR, :], in_=mo0[:])

            last = (OB - 1, HS - HR)
            for ob in range(OB):
                for h0 in range(0, HS, HR):
                    if ob == 0 and h0 < HR:
                        continue
                    is_last = (ob, h0) == last
                    if is_last:
                        mo = po.tile(
                            [128, NP, HR, J], mybir.dt.bfloat16,
                            tag="mo", name="mo"
                        )
                    else:
                        mo = po.tile(
                            [128, HR, NP * J], mybir.dt.bfloat16,
                            tag="mo", name="mo"
                        )
                    for p in range(NP):
                        if is_last and p == NP - 1:
                            continue
                        ps = pp.tile(
                            [128, HR, J], mybir.dt.float32, tag="ps", name="ps"
                        )
                        mm_half(ps, h0, ob, p, 0, first=True, last=False)
                        mm_half(ps, h0, ob, p, 1, first=False, last=True)
                        # In the last tile keep VectorE free for the final
                        # point: its p4 cast goes to ScalarE instead.
                        evac(mo, ps, p, fin=is_last,
                             eng="s" if (is_last and p == 4) else None)
                    if not is_last:
                        nc.sync.dma_start(
                            out=mout[ob, :, h0 : h0 + HR, :], in_=mo[:]
                        )
                    else:
                        # Final point p5 split by rows so only a 1-row group's
                        # cast + DMA sits after the very last matmul. The
                        # rows 0..5 piece goes cast(Scalar) -> DMA(Scalar)
                        # while the last 6 matmuls run; the 1-row piece goes
                        # cast(Vector) -> DMA(Scalar) -- ScalarE is an HWDGE
                        # engine, so nothing queues behind SyncE's backlog.
                        p5 = NP - 1
                        for r0, r1 in ((0, HR - 1), (HR - 1, HR)):
                            ps = pp.tile(
                                [128, HR, J], mybir.dt.float32,
                                tag="ps", name="ps"
                            )
                            sub = ps[:, r0:r1, :]
                            for b in range(CB):
                                for kh in range(KH):
                                    nc.tensor.matmul(
                                        sub,
                                        w_sb[b][ob][:, p5 * KH + kh, :],
                                        x_sb[b][
                                            :,
                                            h0 + kh + r0 : h0 + kh + r1,
                                            p5 * J : (p5 + 1) * J,
                                        ],
                                        start=(b == 0 and kh == 0),
                                        stop=(b == CB - 1 and kh == KH - 1),
                                    )
                            dst = mo[:, p5, r0:r1, :]
                            nc.vector.tensor_copy(out=dst, in_=sub)
                            if r1 == HR:
                                # Only this 1-row piece sits after the last
                                # matmul; ScalarE's HWDGE queue is free.
                                nc.scalar.dma_start(
                                    out=mlast[:, p5, r0:r1], in_=dst
                                )
                            else:
                                nc.sync.dma_start(
                                    out=mlast[:, p5, r0:r1], in_=dst
                                )

    nc.compile()
    return nc


def _to_bf16(a):
    return np.ascontiguousarray(a.astype(ml_dtypes.bfloat16))


def kernel(x: np.ndarray, kernel: np.ndarray) -> np.ndarray:
    global LAST_RESULTS
    if "nc" not in _CACHE:
        _CACHE["nc"] = _build()
    nc = _CACHE["nc"]

    x = np.ascontiguousarray(x, dtype=np.float32)
    g = np.ascontiguousarray(kernel, dtype=np.float32)

    xp = np.pad(x, ((0, 0), (1, 1), (1, 1)))          # [C, H+2, 226]
    # Winograd F(4,3) input transform: 6 point-planes x 56 windows.
    Xt = np.zeros((C, H + 2, NP, J), dtype=np.float32)
    for p in range(NP):
        for i in range(6):
            c = BT[p, i]
            if c:
                Xt[:, :, p, :] += np.float32(c) * xp[:, :, i : 4 * (J - 1) + i + 1 : 4]
    Xt = _to_bf16(Xt.reshape(CB, 128, H + 2, NP * J))

    # Weight transform: Wt[p][o, c, kh] = sum_k G[p,k] g[o,c,kh,k].
    gt = g.transpose(1, 2, 3, 0).astype(np.float64)   # [c, kh, kw, o]
    Wt = np.einsum('pk,chko->cpho', G, gt).astype(np.float32)  # [c, p, kh, o]
    # -> [cb, ob, 128 c, p*3+kh, 128 o]
    w_t = _to_bf16(
        Wt.reshape(CB, 128, NP * KH, OB, 128).transpose(0, 3, 1, 2, 4)
    )

    in_maps = []
    for i in range(N_CORES):
        xs_i = np.ascontiguousarray(Xt[:, :, i * HS : i * HS + HS + 2, :])
        in_maps.append({"xs": xs_i, "w": w_t})

    last_err = None
    for _ in range(3):
        try:
            results = run_bass_kernel_spmd(
                nc, in_maps, core_ids=list(range(N_CORES)), trace=TRACE
            )
            break
        except Exception as e:  # noqa: BLE001
            last_err = e
    else:
        raise last_err
    LAST_RESULTS = results

    # Host output transform: y[4j+m] = sum_p AT[m,p] M[p][j].
    out = np.empty((O, H, W), dtype=np.float32)
    for i, r in enumerate(results.results):
        M = r["mout"].reshape(O, HS, NP, J).astype(np.float32)
        # Fold the p-major last tile back in: mlast [128, NP, HR, J] holds
        # (ob=1, rows HS-HR..HS) for this core.
        M[O - 128 :, HS - HR :, :, :] = (
            r["mlast"].transpose(0, 2, 1, 3).astype(np.float32)
        )
        sl = out[:, i * HS : (i + 1) * HS, :]
        for m in range(4):
            acc = np.zeros((O, HS, J), dtype=np.float32)
            for p in range(NP):
                c = AT[m, p]
                if c:
                    acc += np.float32(c) * M[:, :, p, :]
            sl[:, :, m::4] = acc
    return out


# revision 30
# speedup vs baseline: 1.0440x; 1.0050x over previous
"""Conv2d(256->256, 3x3, pad=1) on 8 TRN2 NeuronCores.

Sharding: data-parallel over output rows (H). Each core computes all 256
output channels for a 28-row slice; weights are replicated (the PE
stationary dim stays a full 128 o-channels either way, but H-sharding
needs no output collectives).

Algorithm: 1D Winograd F(4,3) along W (points {0,+-1,+-2,inf}), direct
3-tap accumulation along H, bf16 matmuls: 6 Winograd points per 4
outputs -> 4.5 MACs/output vs 9 direct. The host applies the B^T input
transform (6 point-planes x 56 stride-4 windows per padded row) and the
G weight transform; the device accumulates, per point p, M[p] =
sum_(c-block, kh) W[p,kh]^T X[p] -- 6 accumulating bf16 matmuls into a
PSUM bank [128 o, 7 h-rows x 56 windows = 392] -- then casts PSUM to
bf16 M planes (Vector/Scalar engines); the host applies A^T. 288
matmuls of N=392 per core = 47 us of PE streaming at 2.4 GHz (vs 504
f32r matmuls with ~190 ns exposed self-weight-loads = ~105 us in the
direct-conv baseline). bf16 rel err ~9.6e-3 on HW (gate 2e-2); fp32
PSUM accumulation, fp64 host transforms.

Schedule (from perfetto/NTFF analysis): engine preamble barrier ends
~7 us and DMA data cannot start flowing before ~8 us (fixed runtime
latencies), streaming at ~0.3 GB/us. The head is therefore
arrival-ordered: w(cb,ob=0) in p-split pieces, x rows in one 9-row piece
per c-block, and tile0 runs its six cb0 half-groups first (PSUM groups
stay open) so the PE can start on ~1.4 MB instead of ~2.7 MB.
Dependency-light warmup matmuls (memset tile) cover 7..12.5 us so the
HAM clock-gate is at 8/8 when real work starts, with filler warmups
between tile0's halves so a slow-DMA run cannot idle the PE >2 us and
re-throttle it. PSUM is evacuated per-p right after each group closes,
alternating Vector/Scalar; the final tile lands p-major in a separate
DRAM tensor (contiguous 784B lines) and its last point is row-split so
only a 1-row cast + ScalarE-issued DMA sits after the last matmul.
"""

import sys

sys.path.insert(0, "/opt/trn_rl_repo")

import numpy as np
import ml_dtypes

import concourse.mybir as mybir
from concourse import bacc
from concourse.tile import TileContext
from concourse.bass_utils import run_bass_kernel_spmd

N_CORES = 8
C, H, W = 256, 224, 224
O = 256
KH = KW = 3
HS = H // N_CORES          # 28 output rows per core
HR = 7                     # output rows per PSUM tile (N = 7*56 = 392)
J = W // 4                 # 56 Winograd windows per row
NP = 6                     # Winograd points per window
CB = C // 128              # c blocks
OB = O // 128              # o blocks

_CACHE = {}
LAST_RESULTS = None
TRACE = False

BT = np.array([
    [4,  0, -5,  0, 1, 0],
    [0, -4, -4,  1, 1, 0],
    [0,  4, -4, -1, 1, 0],
    [0, -2, -1,  2, 1, 0],
    [0,  2, -1, -2, 1, 0],
    [0,  4,  0, -5, 0, 1],
], dtype=np.float64)
G = np.array([
    [1 / 4,      0,     0],
    [-1 / 6, -1 / 6, -1 / 6],
    [-1 / 6,  1 / 6, -1 / 6],
    [1 / 24, 1 / 12,  1 / 6],
    [1 / 24, -1 / 12, 1 / 6],
    [0,          0,     1],
], dtype=np.float64)
AT = np.array([
    [1, 1,  1, 1,  1, 0],
    [0, 1, -1, 2, -2, 0],
    [0, 1,  1, 4,  4, 0],
    [0, 1, -1, 8, -8, 1],
], dtype=np.float64)


def _build():
    nc = bacc.Bacc(None, target_bir_lowering=False)

    xs = nc.dram_tensor(
        "xs", [CB, 128, HS + 2, NP * J], mybir.dt.bfloat16, kind="ExternalInput"
    )
    w = nc.dram_tensor(
        "w", [CB, OB, 128, NP * KH, 128], mybir.dt.bfloat16, kind="ExternalInput"
    )
    mout = nc.dram_tensor(
        "mout", [OB, 128, HS, NP * J], mybir.dt.bfloat16, kind="ExternalOutput"
    )
    # Last tile lands p-major so each per-p piece is contiguous per
    # partition (784B lines); the row-major mout slice would be 112B lines
    # and its ~0.6 MB would take ~6 us after the last matmul.
    mlast = nc.dram_tensor(
        "mlast", [128, NP, HR, J], mybir.dt.bfloat16, kind="ExternalOutput"
    )

    n_warm = 44
    with TileContext(nc) as tc:
        with (
            tc.tile_pool(name="warm", bufs=1) as pwarm,
            tc.tile_pool(name="win", bufs=1) as pw,
            tc.tile_pool(name="xin", bufs=1) as px,
            tc.tile_pool(name="psumw", bufs=1, space="PSUM") as ppw,
            tc.tile_pool(name="psum", bufs=7, space="PSUM") as pp,
            tc.tile_pool(name="outp", bufs=4) as po,
        ):
            # PE warmup: short N=128 matmuls on a memset tile, sized to
            # keep the PE busy until real operands land.
            wt0 = pwarm.tile([128, 128], mybir.dt.bfloat16, tag="warm")
            ps0 = ppw.tile([128, 128], mybir.dt.float32, tag="warmps")
            nc.vector.memset(wt0[:], 0.0)
            for _ in range(n_warm):
                nc.tensor.matmul(ps0[:], wt0[:], wt0[:], start=True, stop=True)

            x_sb = [
                px.tile(
                    [128, HS + 2, NP * J], mybir.dt.bfloat16,
                    tag=f"x{b}", name=f"x{b}"
                )
                for b in range(CB)
            ]
            w_sb = [
                [
                    pw.tile(
                        [128, NP * KH, 128], mybir.dt.bfloat16,
                        tag=f"w{b}{ob}", name=f"w{b}{ob}"
                    )
                    for ob in range(OB)
                ]
                for b in range(CB)
            ]

            def dma_w(b, ob, k0=0, k1=NP * KH, eng=None):
                (eng or nc.sync).dma_start(
                    out=w_sb[b][ob][:, k0:k1, :], in_=w[b, ob, :, k0:k1, :]
                )

            def dma_x(b, r0, r1, eng=None):
                (eng or nc.sync).dma_start(
                    out=x_sb[b][:, r0:r1, :], in_=xs[b, :, r0:r1, :]
                )

            # Head DMAs split across BOTH HWDGE queues (SyncE + ScalarE) --
            # the early per-queue stream rate is the bottleneck, and the two
            # queues drain in parallel. Per-queue issue order matches the
            # cb0-half-first consumption order; weights are p-split so each
            # half-pass starts as soon as its first point's taps are in.
            # ScalarE is otherwise idle until its first cast (~15 us).
            dma_w(0, 0, 0, 9)                      # sync:   w00 p0-2
            dma_x(0, 0, 9, eng=nc.scalar)          # scalar: x0 rows 0-8
            dma_w(0, 0, 9, NP * KH)                # sync:   w00 p3-5
            dma_w(1, 0, 0, 9, eng=nc.scalar)       # scalar: w10 p0-2
            dma_x(1, 0, 9)                         # sync:   x1 rows 0-8
            dma_w(1, 0, 9, NP * KH, eng=nc.scalar)  # scalar: w10 p3-5
            dma_x(0, 9, 16)
            dma_x(1, 9, 16, eng=nc.scalar)
            dma_x(0, 16, 23)
            dma_x(1, 16, 23, eng=nc.scalar)
            dma_x(0, 23, 30)
            dma_x(1, 23, 30, eng=nc.scalar)
            dma_w(0, 1)
            dma_w(1, 1)

            def mm_half(ps, h0, ob, p, b, first, last):
                for kh in range(KH):
                    nc.tensor.matmul(
                        ps[:],
                        w_sb[b][ob][:, p * KH + kh, :],
                        x_sb[b][:, h0 + kh : h0 + kh + HR, p * J : (p + 1) * J],
                        start=(first and kh == 0),
                        stop=(last and kh == KH - 1),
                    )

            def evac(mo, ps, p, fin=False, eng=None):
                if fin:
                    # p-major staging tile: piece is contiguous per partition.
                    dst = mo[:, p]
                else:
                    dst = mo[:, :, p * J : (p + 1) * J]
                if eng is None:
                    eng = "v" if p % 2 == 0 else "s"
                if eng == "v":
                    nc.vector.tensor_copy(out=dst, in_=ps[:])
                else:
                    nc.scalar.copy(out=dst, in_=ps[:])
                if fin:
                    nc.sync.dma_start(out=mlast[:, p], in_=mo[:, p])

            # First tile: all six cb0 half-groups first (needs only x rows
            # 0..8 of cb0 + w(0,0)), then the cb1 halves.
            mo0 = po.tile([128, HR, NP * J], mybir.dt.bfloat16, tag="mo", name="mo")
            ps0l = []
            for p in range(NP):
                ps = pp.tile([128, HR, J], mybir.dt.float32, tag="ps", name="ps")
                ps0l.append(ps)
                mm_half(ps, 0, 0, p, 0, first=True, last=False)
            # Filler warmups: if cb1's operands are still in flight, these
            # keep the PE busy through the gap so the HAM clock-gate cannot
            # re-throttle (observed: a ~2 us idle here dropped it to 4/8 and
            # the next ~15 matmuls ran at 1.2 GHz).
            for _ in range(12):
                nc.tensor.matmul(ps0[:], wt0[:], wt0[:], start=True, stop=True)
            for p in range(NP):
                ps = ps0l[p]
                mm_half(ps, 0, 0, p, 1, first=False, last=True)
                evac(mo0, ps, p)
            nc.sync.dma_start(out=mout[0, :, 0:HR, :], in_=mo0[:])

            last = (OB - 1, HS - HR)
            for ob in range(OB):
                for h0 in range(0, HS, HR):
                    if ob == 0 and h0 < HR:
                        continue
                    is_last = (ob, h0) == last
                    if is_last:
                        mo = po.tile(
                            [128, NP, HR, J], mybir.dt.bfloat16,
                            tag="mo", name="mo"
                        )
                    else:
                        mo = po.tile(
                            [128, HR, NP * J], mybir.dt.bfloat16,
                            tag="mo", name="mo"
                        )
                    for p in range(NP):
                        if is_last and p == NP - 1:
                            continue
                        ps = pp.tile(
                            [128, HR, J], mybir.dt.float32, tag="ps", name="ps"
                        )
                        mm_half(ps, h0, ob, p, 0, first=True, last=False)
                        mm_half(ps, h0, ob, p, 1, first=False, last=True)
                        # In the last tile keep VectorE free for the final
                        # point: its p4 cast goes to ScalarE instead.
                        evac(mo, ps, p, fin=is_last,
                             eng="s" if (is_last and p == 4) else None)
                    if not is_last:
                        nc.sync.dma_start(
                            out=mout[ob, :, h0 : h0 + HR, :], in_=mo[:]
                        )
                    else:
                        # Final point p5 split by rows so only a 1-row group's
                        # cast + DMA sits after the very last matmul. The
                        # rows 0..5 piece goes cast(Scalar) -> DMA(Scalar)
                        # while the last 6 matmuls run; the 1-row piece goes
                        # cast(Vector) -> DMA(Scalar) -- ScalarE is an HWDGE
                        # engine, so nothing queues behind SyncE's backlog.
                        p5 = NP - 1
                        for r0, r1 in ((0, HR - 1), (HR - 1, HR)):
                            ps = pp.tile(
                                [128, HR, J], mybir.dt.float32,
                                tag="ps", name="ps"
                            )
                            sub = ps[:, r0:r1, :]
                            for b in range(CB):
                                for kh in range(KH):
                                    nc.tensor.matmul(
                                        sub,
                                        w_sb[b][ob][:, p5 * KH + kh, :],
                                        x_sb[b][
                                            :,
                                            h0 + kh + r0 : h0 + kh + r1,
                                            p5 * J : (p5 + 1) * J,
                                        ],
                                        start=(b == 0 and kh == 0),
                                        stop=(b == CB - 1 and kh == KH - 1),
                                    )
                            dst = mo[:, p5, r0:r1, :]
                            nc.vector.tensor_copy(out=dst, in_=sub)
                            if r1 == HR:
                                # Only this 1-row piece sits after the last
                                # matmul; ScalarE's HWDGE queue is free.
                                nc.scalar.dma_start(
                                    out=mlast[:, p5, r0:r1], in_=dst
                                )
                            else:
                                nc.sync.dma_start(
                                    out=mlast[:, p5, r0:r1], in_=dst
                                )

    nc.compile()
    return nc


def _to_bf16(a):
    return np.ascontiguousarray(a.astype(ml_dtypes.bfloat16))


def kernel(x: np.ndarray, kernel: np.ndarray) -> np.ndarray:
    global LAST_RESULTS
    if "nc" not in _CACHE:
        _CACHE["nc"] = _build()
    nc = _CACHE["nc"]

    x = np.ascontiguousarray(x, dtype=np.float32)
    g = np.ascontiguousarray(kernel, dtype=np.float32)

    xp = np.pad(x, ((0, 0), (1, 1), (1, 1)))          # [C, H+2, 226]
    # Winograd F(4,3) input transform: 6 point-planes x 56 windows.
    Xt = np.zeros((C, H + 2, NP, J), dtype=np.float32)
    for p in range(NP):
        for i in range(6):
            c = BT[p, i]
            if c:
                Xt[:, :, p, :] += np.float32(c) * xp[:, :, i : 4 * (J - 1) + i + 1 : 4]
    Xt = _to_bf16(Xt.reshape(CB, 128, H + 2, NP * J))

    # Weight transform: Wt[p][o, c, kh] = sum_k G[p,k] g[o,c,kh,k].
    gt = g.transpose(1, 2, 3, 0).astype(np.float64)   # [c, kh, kw, o]
    Wt = np.einsum('pk,chko->cpho', G, gt).astype(np.float32)  # [c, p, kh, o]
    # -> [cb, ob, 128 c, p*3+kh, 128 o]
    w_t = _to_bf16(
        Wt.reshape(CB, 128, NP * KH, OB, 128).transpose(0, 3, 1, 2, 4)
    )

    in_maps = []
    for i in range(N_CORES):
        xs_i = np.ascontiguousarray(Xt[:, :, i * HS : i * HS + HS + 2, :])
        in_maps.append({"xs": xs_i, "w": w_t})

    last_err = None
    for _ in range(3):
        try:
            results = run_bass_kernel_spmd(
                nc, in_maps, core_ids=list(range(N_CORES)), trace=TRACE
            )
            break
        except Exception as e:  # noqa: BLE001
            last_err = e
    else:
        raise last_err
    LAST_RESULTS = results

    # Host output transform: y[4j+m] = sum_p AT[m,p] M[p][j].
    out = np.empty((O, H, W), dtype=np.float32)
    for i, r in enumerate(results.results):
        M = r["mout"].reshape(O, HS, NP, J).astype(np.float32)
        # Fold the p-major last tile back in: mlast [128, NP, HR, J] holds
        # (ob=1, rows HS-HR..HS) for this core.
        M[O - 128 :, HS - HR :, :, :] = (
            r["mlast"].transpose(0, 2, 1, 3).astype(np.float32)
        )
        sl = out[:, i * HS : (i + 1) * HS, :]
        for m in range(4):
            acc = np.zeros((O, HS, J), dtype=np.float32)
            for p in range(NP):
                c = AT[m, p]
                if c:
                    acc += np.float32(c) * M[:, :, p, :]
            sl[:, :, m::4] = acc
    return out
